# revision 28
# baseline (speedup 1.0000x reference)
"""Trainium2 Bass kernel for CausalFlowModel (RNN scan + 2 MLPs + combinator).

Sharding: data-parallel over batch across 8 NeuronCores (64 rows/core).
All weights replicated, pre-transposed+packed on host into lhsT tile banks.
Everything on-device runs in bf16 with fp32 PSUM accumulation; biases are
folded into the matmuls via an appended ones-row (they are all zero for this
problem, but handled correctly anyway).

Layout convention: all activations live TRANSPOSED in SBUF as
[feature-partition, batch-column] so the 511-step recurrence needs no
per-step transposes:  hT_{t+1}[m-block] = tanh( sum_k WhT[k,m].T @ hT_t[k]
                                               + WuT_aug[m].T @ uT_aug_t )

RNN step schedule (the performance-critical part): each step's pre-activation
accumulates into TWO PSUM banks (bank A = h-regions 0..1, bank B = 2..3) so
the two tanh halves on ScalarE can each overlap TensorE work on the OTHER
bank (Tile serializes same-bank PE-writes vs ACT-reads).  Matmul slot order
per step:
    u0(A) u1(A) u2(B) u3(B)                # independent of h, fills latency
    (m01,k01 -> A) (m23,k01 -> B)          # consume prev blocks 0,1 only
    (m01,k23 -> A)  [tanh A]  (m23,k23 -> B)  [tanh B]
Bank A finishes 4 slots early, so tanh(A) hides under the B tail + next
step's u-matmuls; tanh(B) hides under the next step's 12 non-k23 slots.
"""

import numpy as np
import ml_dtypes

B, T = 512, 512
SD, CD, H = 256, 64, 512
D1, D2 = 1024, 1024
NCORES = 8
BL = B // NCORES          # 64 batch rows per core
CHUNK = 64                # u steps per DMA chunk
WARMUP_MM = 200           # dense dummy matmuls: ~10.6us at the 1.2GHz boot
                          # clock, the measured threshold for a COLD device
                          # to flip to 2.4GHz before the RNN starts
XDNN_T0 = 416             # RNN step where interleaved x_dnn work begins
NSTEPS = T - 1            # 511 scan steps

_BF = ml_dtypes.bfloat16

_CACHE = {}


def _bf16(a):
    return np.ascontiguousarray(np.asarray(a, np.float32)).astype(_BF)


def _pack_kxm(W, n_m, n_k, k_off=0):
    """lhsT tile bank [128, n_k*n_m*128]; block j=k*n_m+m is
    W[m*128:(m+1)*128, k_off+k*128 : k_off+(k+1)*128].T"""
    cols = []
    for k in range(n_k):
        for m in range(n_m):
            cols.append(W[m * 128:(m + 1) * 128,
                          k_off + k * 128: k_off + (k + 1) * 128].T)
    return np.concatenate(cols, axis=1)


def _pack_head_bias(W, bvec, n_m, width):
    """[width+1, n_m*128]; block m = [W[m*128:(m+1)*128, :width].T ; b[mblock]]"""
    cols = []
    for m in range(n_m):
        blk = np.concatenate(
            [W[m * 128:(m + 1) * 128, :width].T,
             bvec[m * 128:(m + 1) * 128][None, :]], axis=0)
        cols.append(blk)
    return np.concatenate(cols, axis=1)


def _weight_arrays(inp):
    i2h_W, i2h_b = inp["i2h_W"], inp["i2h_b"]
    w = {
        "whT": _pack_kxm(i2h_W, 4, 4, k_off=CD),
        "wuT": _pack_head_bias(i2h_W, i2h_b, 4, CD),          # [65, 512]
        "x1T": _pack_kxm(inp["x1_W"], 8, 2, k_off=1),
        "x1tb": _pack_head_bias(inp["x1_W"], inp["x1_b"], 8, 1),  # [2, 1024]
        "x2T": _pack_kxm(inp["x2_W"], 8, 8),
        "x2b": np.asarray(inp["x2_b"], np.float32)[None, :],
        "x3T": _pack_kxm(inp["x3_W"], 2, 8),
        "x3b": np.asarray(inp["x3_b"], np.float32)[None, :],
        "u1T": _pack_kxm(inp["u1_W"], 8, 2, k_off=1),
        "u1tb": _pack_head_bias(inp["u1_W"], inp["u1_b"], 8, 1),
        "u2T": _pack_kxm(inp["u2_W"], 8, 8),
        "u2b": np.asarray(inp["u2_b"], np.float32)[None, :],
        "u3T": _pack_kxm(inp["u3_W"], 2, 8),
        "u3b": np.asarray(inp["u3_b"], np.float32)[None, :],
        "h2oT": _pack_kxm(inp["h2o_W"], 2, 4, k_off=CD),
        "h2o_uT": _pack_head_bias(inp["h2o_W"], inp["h2o_b"], 2, CD),  # [65, 256]
        "combT": _pack_kxm(inp["comb_W"], 2, 4),
        "combb": np.asarray(inp["comb_b"], np.float32)[None, :],
    }
    return {k: _bf16(v) for k, v in w.items()}


def _per_core_arrays(inp, c):
    t = np.asarray(inp["t"], np.float32)
    x = np.asarray(inp["x"], np.float32)
    u = np.asarray(inp["u"], np.float32)
    b0 = c * BL
    us = u[:, b0:b0 + BL, :].transpose(2, 0, 1).reshape(CD, T * BL)
    u_aug = np.concatenate([us, np.ones((1, T * BL), np.float32)], axis=0)
    xT = x[b0:b0 + BL].T                              # [256, BL]
    xt = np.concatenate([xT[:128], xT[128:]], axis=1)  # [128, 2*BL]
    tb = np.stack([t[b0:b0 + BL, 0], np.ones(BL, np.float32)], axis=0)  # [2, BL]
    return {"u_aug": _bf16(u_aug), "xt": _bf16(xt), "tb": _bf16(tb)}


def _build_program(debug=False):
    import concourse.bass as bass
    import concourse.mybir as mybir
    from concourse import bacc
    from concourse.tile import TileContext

    bf = mybir.dt.bfloat16
    f32 = mybir.dt.float32
    TANH = mybir.ActivationFunctionType.Tanh

    nc = bacc.Bacc("TRN2", target_bir_lowering=False, debug=False)

    d_in = {}
    def din(name, shape, dt=bf):
        d_in[name] = nc.dram_tensor(name, list(shape), dt, kind="ExternalInput")
        return d_in[name]

    u_aug_d = din("u_aug", (CD + 1, T * BL))
    xt_d = din("xt", (128, 2 * BL))
    tb_d = din("tb", (2, BL))
    wh_d = din("whT", (128, 16 * 128))
    wu_d = din("wuT", (CD + 1, 4 * 128))
    x1_d = din("x1T", (128, 16 * 128))
    x1tb_d = din("x1tb", (2, 8 * 128))
    x2_d = din("x2T", (128, 64 * 128))
    x2b_d = din("x2b", (1, 8 * 128))
    x3_d = din("x3T", (128, 16 * 128))
    x3b_d = din("x3b", (1, 2 * 128))
    u1_d = din("u1T", (128, 16 * 128))
    u1tb_d = din("u1tb", (2, 8 * 128))
    u2_d = din("u2T", (128, 64 * 128))
    u2b_d = din("u2b", (1, 8 * 128))
    u3_d = din("u3T", (128, 16 * 128))
    u3b_d = din("u3b", (1, 2 * 128))
    h2o_d = din("h2oT", (128, 8 * 128))
    h2ou_d = din("h2o_uT", (CD + 1, 2 * 128))
    comb_d = din("combT", (128, 8 * 128))
    combb_d = din("combb", (1, 2 * 128))
    out_d = nc.dram_tensor("out", [2 * 128, BL], f32, kind="ExternalOutput")
    dbg = {}
    if debug:
        for name in ("dbg_h0", "dbg_h1", "dbg_hlast"):
            dbg[name] = nc.dram_tensor(name, [128, 4 * BL], f32,
                                       kind="ExternalOutput")
        for name in ("dbg_r", "dbg_s", "dbg_c"):
            dbg[name] = nc.dram_tensor(name, [128, 2 * BL], f32,
                                       kind="ExternalOutput")

    with TileContext(nc) as tc:
        with (
            tc.tile_pool(name="consts", bufs=1) as consts,
            tc.tile_pool(name="upool", bufs=2) as upool,
            tc.tile_pool(name="hpool", bufs=6) as hpool,
            tc.tile_pool(name="work", bufs=1) as work,
        ):
            mm = nc.tensor.matmul
            # --- PE p-state warmup: dense dummy stream overlapping the ---
            # --- initial DMA wait; pushes the PE clock to 2.4GHz early ---
            warm_ctx = tc.tile_pool(name="warmps", bufs=1, space="PSUM")
            warmps = warm_ctx.__enter__()
            dummy = work.tile([128, 128], bf, name="dummy")
            nc.vector.memset(dummy[:, :], 0.0)
            wps = warmps.tile([128, 64], f32, name="wps")
            for _ in range(WARMUP_MM):
                mm(wps[:, :], dummy[:, :], dummy[:, 0:64],
                   start=True, stop=True, skip_group_check=True)

            def cload(dram, shape, dt=bf, name=None):
                tile = consts.tile(list(shape), dt, name=name)
                nc.sync.dma_start(out=tile[:, :], in_=dram[:, :])
                return tile

            # --- DMAs the RNN needs first, ordered so the scan starts ASAP:
            # a small head of u-chunk 0, then wu, then wh k-slices, then the
            # rest of chunk 0 ---
            u_tile = upool.tile([CD + 1, CHUNK * BL], bf, name="ut")
            head = 8 * BL
            nc.sync.dma_start(out=u_tile[:, 0:head], in_=u_aug_d[:, 0:head])
            wu_sb = cload(wu_d, (CD + 1, 4 * 128), name="wu_sb")
            wh_sb = consts.tile([128, 16 * 128], bf, name="wh_sb")
            for kk in range(4):
                nc.sync.dma_start(out=wh_sb[:, kk * 512:(kk + 1) * 512],
                                  in_=wh_d[:, kk * 512:(kk + 1) * 512])
            nc.sync.dma_start(out=u_tile[:, head:CHUNK * BL],
                              in_=u_aug_d[:, head:CHUNK * BL])
            # --- remaining consts: DMAs deferred into the RNN loop (one
            # per 3rd step from step 28) -- none needed before step ~416,
            # and the upfront 6.5MB burst was stalling RNN steps 4-23 ---
            dqueue = []

            def dload(dram, shape, name):
                tile = consts.tile(list(shape), bf, name=name)
                dqueue.append((tile[:, :], dram[:, :]))
                return tile
            tb_sb = dload(tb_d, (2, BL), "tb_sb")
            ones_sb = consts.tile([1, BL], bf, name="ones_sb")
            dqueue.append((ones_sb[:, :], tb_d[1:2, :]))
            xt_sb = dload(xt_d, (128, 2 * BL), "xt_sb")
            h2o_sb = dload(h2o_d, (128, 8 * 128), "h2o_sb")
            h2ou_sb = dload(h2ou_d, (CD + 1, 2 * 128), "h2ou_sb")
            x1_sb = dload(x1_d, (128, 16 * 128), "x1_sb")
            x1tb_sb = dload(x1tb_d, (2, 8 * 128), "x1tb_sb")
            x2_sb = dload(x2_d, (128, 64 * 128), "x2_sb")
            x2b_sb = dload(x2b_d, (1, 8 * 128), "x2b_sb")
            x3_sb = dload(x3_d, (128, 16 * 128), "x3_sb")
            x3b_sb = dload(x3b_d, (1, 2 * 128), "x3b_sb")
            u1_sb = dload(u1_d, (128, 16 * 128), "u1_sb")
            u1tb_sb = dload(u1tb_d, (2, 8 * 128), "u1tb_sb")
            u2_sb = dload(u2_d, (128, 64 * 128), "u2_sb")
            u2b_sb = dload(u2b_d, (1, 8 * 128), "u2b_sb")
            u3_sb = dload(u3_d, (128, 16 * 128), "u3_sb")
            u3b_sb = dload(u3b_d, (1, 2 * 128), "u3b_sb")
            comb_sb = dload(comb_d, (128, 8 * 128), "comb_sb")
            combb_sb = dload(combb_d, (1, 2 * 128), "combb_sb")

            warm_ctx.__exit__(None, None, None)
            mlpps_ctx = tc.tile_pool(name="mlpps", bufs=2, space="PSUM")
            mlpps = mlpps_ctx.__enter__()

            def mlptile():
                return mlpps.tile([128, 4 * BL], f32, name="mlp")

            rnnps_ctx = tc.tile_pool(name="rnnps", bufs=3, space="PSUM")
            rnnps = rnnps_ctx.__enter__()

            # ---------------- RNN scan: 511 steps ----------------
            # The u-part matmuls for step t+2 are emitted at the END of
            # iteration t (explicit 2-deep software pipeline): they are the
            # only h-independent PE work, and placing them right after each
            # step's tail keeps the PE busy while tanh(A)/tanh(B) of the
            # previous step complete.  h-slot order gives each tanh half
            # ~10 slots of downstream fill before its next-step consumers.
            from concourse.tile import add_dep_helper
            rnn_ps = {}

            def emit_u(t, after=None):
                uc = (t % CHUNK) * BL
                urhs = u_tiles[t // CHUNK][:, uc:uc + BL]
                ps_a = rnnps.tile([128, 2 * BL], f32, name="ps_a")
                ps_b = rnnps.tile([128, 2 * BL], f32, name="ps_b")
                rnn_ps[t] = (ps_a, ps_b)
                for m in range(4):
                    o = (ps_a, ps_a, ps_b, ps_b)[m][:, BL * (m % 2):
                                                    BL * (m % 2 + 1)]
                    inst = mm(o, wu_sb[:, 128 * m:128 * (m + 1)], urhs,
                              start=(m % 2 == 0), stop=(t == 0),
                              skip_group_check=True)
                    if after is not None:
                        add_dep_helper(inst.ins, after.ins, sync=False,
                                       reason="pin u-fill to period tail")

            # ---- x_dnn (state MLP) work queue, drained into the idle PE/ACT
            # slots of RNN steps >= XDNN_T0: items are ('mm', fn) emitted
            # after a step's u-fill, or ('act', fn) emitted right after a
            # step's tanh(B) where the ACT engine has ~480ns of idle. ----
            xwork = []
            xst = {}

            def _xl1_mms(half):
                def f():
                    p = xst.setdefault(f"p1{half}", mlptile())
                    in_blocks = [xt_sb[:, 0:BL], xt_sb[:, BL:2 * BL]]
                    for mi in range(4):
                        m = half * 4 + mi
                        o = p[:, BL * mi:BL * (mi + 1)]
                        mm(o, x1tb_sb[:, 128 * m:128 * (m + 1)], tb_sb[:, :],
                           start=(mi == 0), stop=False, skip_group_check=True)
                        for k in range(2):
                            j = k * 8 + m
                            mm(o, x1_sb[:, 128 * j:128 * (j + 1)],
                               in_blocks[k], start=False, stop=(k == 1),
                               skip_group_check=True)
                return f

            def _xact(src_key, dst_key, dst_shape, q):
                def f():
                    dst = xst.setdefault(dst_key,
                                         work.tile([128, dst_shape], bf,
                                                   name=dst_key))
                    nc.scalar.activation(
                        dst[:, q * 2 * BL:(q + 1) * 2 * BL],
                        xst[src_key][:, (q % 2) * 2 * BL:
                                     (q % 2 + 1) * 2 * BL], TANH)
                return f

            def _xl2_mms(half, mi):
                def f():
                    p = xst.setdefault(f"p2{half}", mlptile())
                    m = half * 4 + mi
                    o = p[:, BL * mi:BL * (mi + 1)]
                    mm(o, x2b_sb[:, 128 * m:128 * (m + 1)], ones_sb[:, :],
                       start=(mi == 0), stop=False, skip_group_check=True)
                    for k in range(8):
                        j = k * 8 + m
                        mm(o, x2_sb[:, 128 * j:128 * (j + 1)],
                           xst["xa1"][:, BL * k:BL * (k + 1)],
                           start=False, stop=(k == 7), skip_group_check=True)
                return f

            def _xl3_mms(m):
                def f():
                    p = xst.setdefault("p3", mlptile())
                    o = p[:, BL * m:BL * (m + 1)]
                    mm(o, x3b_sb[:, 128 * m:128 * (m + 1)], ones_sb[:, :],
                       start=(m == 0), stop=False, skip_group_check=True)
                    for k in range(8):
                        j = k * 2 + m
                        mm(o, x3_sb[:, 128 * j:128 * (j + 1)],
                           xst["xa2"][:, BL * k:BL * (k + 1)],
                           start=False, stop=(k == 7), skip_group_check=True)
                return f

            def _xcopy():
                s_t = work.tile([128, 2 * BL], bf, name="s_sb")
                xst["s_sb"] = s_t
                nc.vector.tensor_copy(s_t[:, :], xst["p3"][:, 0:2 * BL])

            for half in (0, 1):
                xwork.append(('mm', _xl1_mms(half)))        # 12 mms each
            for half in (0, 1):
                for q in (0, 1):
                    xwork.append(('act', _xact(f"p1{half}", "xa1", 8 * BL,
                                               half * 2 + q)))
            for half in (0, 1):
                for mi in range(4):
                    xwork.append(('mm', _xl2_mms(half, mi)))  # 9 mms each
            for half in (0, 1):
                for q in (0, 1):
                    xwork.append(('act', _xact(f"p2{half}", "xa2", 8 * BL,
                                               half * 2 + q)))
            for m in (0, 1):
                xwork.append(('mm', _xl3_mms(m)))             # 9 mms each
            xwork.append(('mm', _xcopy))

            u_tiles = {0: u_tile}
            emit_u(0)
            emit_u(1)
            hcur = None
            hnext = hpool.tile([128, 4 * BL], bf, name="h")
            for t in range(NSTEPS):
                tpre = t + 8
                if tpre % CHUNK == 0 and tpre <= NSTEPS - 1:
                    nt = upool.tile([CD + 1, CHUNK * BL], bf, name="ut")
                    nc.sync.dma_start(
                        out=nt[:, :],
                        in_=u_aug_d[:, tpre * BL:(tpre + CHUNK) * BL])
                    u_tiles[tpre // CHUNK] = nt
                    u_tiles.pop(tpre // CHUNK - 2, None)
                ps_a, ps_b = rnn_ps.pop(t)
                psb = (ps_a, ps_a, ps_b, ps_b)

                def reg(m):
                    return psb[m][:, BL * (m % 2):BL * (m % 2 + 1)]

                hnew = hnext
                last_h = None
                if t > 0:
                    def hmm(m, k):
                        return mm(reg(m), wh_sb[:, 128 * (k * 4 + m):
                                                128 * (k * 4 + m + 1)],
                                  hcur[:, BL * k:BL * (k + 1)],
                                  start=False, stop=(k == 3),
                                  skip_group_check=True)
                    # slots: k01A(4) k01B(2) k23A(4) [tanh A]
                    #        k01B(2) k23B(4) [tanh B]  u(t+2) x4
                    for m, k in ((0, 0), (1, 0), (0, 1), (1, 1),
                                 (2, 0), (3, 0),
                                 (0, 2), (0, 3), (1, 2), (1, 3)):
                        hmm(m, k)
                    nc.scalar.activation(hnew[:, 0:2 * BL], ps_a[:, :], TANH)
                    for m, k in ((2, 1), (3, 1),
                                 (2, 2), (2, 3), (3, 2), (3, 3)):
                        last_h = hmm(m, k)
                else:
                    nc.scalar.activation(hnew[:, 0:2 * BL], ps_a[:, :], TANH)
                nc.scalar.activation(hnew[:, 2 * BL:4 * BL], ps_b[:, :], TANH)
                if t >= XDNN_T0 and xwork and xwork[0][0] == 'act':
                    xwork.pop(0)[1]()
                tn = t + 2
                if tn <= NSTEPS - 1:
                    emit_u(tn, after=last_h)
                if t >= 28 and t % 3 == 1 and dqueue:
                    o_ap, i_ap = dqueue.pop(0)
                    nc.sync.dma_start(out=o_ap, in_=i_ap)
                if t >= XDNN_T0 and xwork and xwork[0][0] == 'mm':
                    xwork.pop(0)[1]()
                hnext = hpool.tile([128, 4 * BL], bf, name="h")
                hcur = hnew
                if debug and t in (0, 1):
                    nc.gpsimd.dma_start(out=dbg[f"dbg_h{t}"][:, :],
                                        in_=hcur[:, :])
            if debug:
                nc.gpsimd.dma_start(out=dbg["dbg_hlast"][:, :], in_=hcur[:, :])
            rnnps_ctx.__exit__(None, None, None)
            # dummy filler pool: keeps the PE stream dense through the tail's
            # tanh-latency gaps so the clock does not drop to the mid p-state
            fill_ctx = tc.tile_pool(name="fillps", bufs=1, space="PSUM")
            fillps = fill_ctx.__enter__()
            fps = fillps.tile([128, 64], f32, name="fps")

            def pefill(n):
                for _ in range(n):
                    mm(fps[:, :], dummy[:, :], dummy[:, 0:64],
                       start=True, stop=True, skip_group_check=True)

            # ---------------- h2o: r = tanh(h2o_W @ [u_last; h_last] + b) ----
            uc_last = ((T - 1) % CHUNK) * BL
            u_last_tile = u_tiles[(T - 1) // CHUNK]
            ps = mlptile()
            for m in range(2):
                mm(ps[:, BL * m:BL * (m + 1)],
                   h2ou_sb[:, 128 * m:128 * (m + 1)],
                   u_last_tile[:, uc_last:uc_last + BL], start=(m == 0),
                   stop=False, skip_group_check=True)
                for k in range(4):
                    j = k * 2 + m
                    mm(ps[:, BL * m:BL * (m + 1)],
                       h2o_sb[:, 128 * j:128 * (j + 1)],
                       hcur[:, BL * k:BL * (k + 1)],
                       start=False, stop=(k == 3), skip_group_check=True)
            r_sb = work.tile([128, 2 * BL], bf, name="r_sb")

            # ---------------- two MLPs, interleaved layer-by-layer ----------
            # PE order: h2o | xL1 | uL1 | xL2 | uL2 | xL3 | uL3 | comb;
            # ACT order: r | xL1 tanhs | uL1 | xL2 | uL2 -- each MLP's tanh
            # latency hides under the other MLP's matmuls.
            def l1_mms(w1_sb, w1tb_sb, in_blocks):
                pss = []
                for half in range(2):
                    p = mlptile()
                    for mi in range(4):
                        m = half * 4 + mi
                        o = p[:, BL * mi:BL * (mi + 1)]
                        mm(o, w1tb_sb[:, 128 * m:128 * (m + 1)], tb_sb[:, :],
                           start=(mi == 0), stop=False, skip_group_check=True)
                        for k in range(2):
                            j = k * 8 + m
                            mm(o, w1_sb[:, 128 * j:128 * (j + 1)],
                               in_blocks[k], start=False, stop=(k == 1),
                               skip_group_check=True)
                    pss.append(p)
                return pss

            def l2_mms(w2_sb, w2b_sb, a1):
                pss = []
                for half in range(2):
                    p = mlptile()
                    for mi in range(4):
                        m = half * 4 + mi
                        o = p[:, BL * mi:BL * (mi + 1)]
                        mm(o, w2b_sb[:, 128 * m:128 * (m + 1)], ones_sb[:, :],
                           start=(mi == 0), stop=False, skip_group_check=True)
                        for k in range(8):
                            j = k * 8 + m
                            mm(o, w2_sb[:, 128 * j:128 * (j + 1)],
                               a1[:, BL * k:BL * (k + 1)],
                               start=False, stop=(k == 7),
                               skip_group_check=True)
                    pss.append(p)
                return pss

            def l3_mms(w3_sb, w3b_sb, a2):
                p = mlptile()
                for m in range(2):
                    o = p[:, BL * m:BL * (m + 1)]
                    mm(o, w3b_sb[:, 128 * m:128 * (m + 1)], ones_sb[:, :],
                       start=(m == 0), stop=False, skip_group_check=True)
                    for k in range(8):
                        j = k * 2 + m
                        mm(o, w3_sb[:, 128 * j:128 * (j + 1)],
                           a2[:, BL * k:BL * (k + 1)],
                           start=False, stop=(k == 7), skip_group_check=True)
                return p

            def act4(dst, pss):
                for half in range(2):
                    for q in range(2):
                        nc.scalar.activation(
                            dst[:, (half * 2 + q) * 2 * BL:
                                (half * 2 + q + 1) * 2 * BL],
                            pss[half][:, q * 2 * BL:(q + 1) * 2 * BL], TANH)

            nc.scalar.activation(r_sb[:, 0:BL], ps[:, 0:BL], TANH)
            nc.scalar.activation(r_sb[:, BL:2 * BL], ps[:, BL:2 * BL], TANH)
            pefill(24)
            ups1 = l1_mms(u1_sb, u1tb_sb,
                          [r_sb[:, 0:BL], r_sb[:, BL:2 * BL]])
            ua1 = work.tile([128, 8 * BL], bf, name="ua1")
            act4(ua1, ups1)
            pefill(36)
            ups2 = l2_mms(u2_sb, u2b_sb, ua1)
            ua2 = work.tile([128, 8 * BL], bf, name="ua2")
            act4(ua2, ups2)
            pefill(30)
            ups3 = l3_mms(u3_sb, u3b_sb, ua2)
            s_sb = xst["s_sb"]
            c_sb = work.tile([128, 2 * BL], bf, name="c_sb")
            nc.vector.tensor_copy(c_sb[:, :], ups3[:, 0:2 * BL])
            pefill(12)

            # ---------------- combinator ----------------
            ps = mlptile()
            for m in range(2):
                o = ps[:, BL * m:BL * (m + 1)]
                mm(o, combb_sb[:, 128 * m:128 * (m + 1)], ones_sb[:, :],
                   start=(m == 0), stop=False, skip_group_check=True)
                for k in range(4):
                    j = k * 2 + m
                    rhs = (s_sb[:, BL * k:BL * (k + 1)] if k < 2
                           else c_sb[:, BL * (k - 2):BL * (k - 1)])
                    mm(o, comb_sb[:, 128 * j:128 * (j + 1)], rhs,
                       start=False, stop=(k == 3), skip_group_check=True)
            out_sb = work.tile([128, 2 * BL], f32, name="out_sb")
            nc.vector.tensor_copy(out_sb[:, :], ps[:, 0:2 * BL])
            nc.sync.dma_start(out=out_d[0:128, :], in_=out_sb[:, 0:BL])
            nc.sync.dma_start(out=out_d[128:256, :], in_=out_sb[:, BL:2 * BL])
            fill_ctx.__exit__(None, None, None)
            mlpps_ctx.__exit__(None, None, None)

    nc.compile()
    return nc


def _get_program():
    if "nc" not in _CACHE:
        _CACHE["nc"] = _build_program()
    return _CACHE["nc"]


def run(inputs, trace=False, trace_cores=None):
    from concourse.bass_utils import run_bass_kernel_spmd

    nc = _get_program()
    w = _weight_arrays(inputs)
    in_maps = []
    for c in range(NCORES):
        m = dict(w)
        m.update(_per_core_arrays(inputs, c))
        in_maps.append(m)
    res = run_bass_kernel_spmd(nc, in_maps, list(range(NCORES)),
                               trace=trace, trace_cores=trace_cores)
    out = np.empty((B, SD), np.float32)
    for c in range(NCORES):
        out[c * BL:(c + 1) * BL, :] = np.asarray(res.results[c]["out"]).T
    return out, res


def kernel(**inputs):
    out, _ = run(inputs)
    return out

